# revision 26
# baseline (speedup 1.0000x reference)
"""Causal self-attention kernel for Trainium2, 8 NeuronCores.

Sharding: DP4 x TP2. Core c = 2*b + g handles batch b (2048 tokens) and
head-group g (8 of 16 heads). Per core:
  - x arrives HOST-pretransposed to [d_model, tokens] bf16, so startup is
    plain streaming DMAs (no xbar transposes, no serialization),
  - QKV matmuls in bf16: Q,K dim-major ([head_dim, tokens]), V token-major
    with a ones column at 64 (softmax denominator for free),
  - attention per head pair, software-pipelined: score unit S(c) one step
    ahead of AV unit A(c), so the PE streams score(c+1) while ACT exps
    score(c); both heads' 64-contraction score matmuls sit in different PE
    row groups (concurrent); causal handling by skipping fully-masked
    tiles, narrowing exp/AV to the live column range, and a 0/1 mask
    multiply on the 128-wide diagonal band,
  - epilogue copies the PSUM y accumulator to SBUF immediately (frees the
    single-buffered PSUM bank), then normalizes via reciprocal_approx_fast
    + gpsimd partition_broadcast into bf16 yT,
  - per (q-tile, head-pair) bf16 AllGather of normalized y between the TP
    pair (16 small AGs, each hidden under later attention),
  - local projection from the gathered full-1024-dim y onto this core's
    512 output columns (bf16), host concatenates the column halves.

QKV chains for token tile n+1 and projection for tile n-1 are zippered
into attention tile n so the PE always has independent matmuls while ACT
drains the exps.

Everything (shapes, sharding) is hardcoded for
x: [4, 2048, 1024], w_qkv: [1024, 3072], w_proj: [1024, 1024], f32.
"""

import ml_dtypes
import numpy as np

import concourse.bacc as bacc
import concourse.mybir as mybir
import concourse.tile as tile
from concourse.bass_utils import run_bass_kernel_spmd

F32 = mybir.dt.float32
BF16 = mybir.dt.bfloat16

S = 2048  # tokens per core (one batch element)
D = 1024  # d_model
HL = 8  # heads per core (local)
HD = 64  # head dim
GD = HL * HD  # 512, head-group dim
NQT = S // 512  # 4 q-tiles of 512
NDM = D // 128  # 8 d_model chunks
NTOK = S // 128  # 16 token tiles of 128
VW = 72  # v_sb inner stride (64 hd + ones + pad)

_NC_CACHE = {}


def _qkv_units(nc, P, n):
    """QKV matmul half-chains for token tile n (512 tokens), as units."""
    if n >= NQT:
        return []
    units = []

    def qk_half(m, half):
        def emit():
            if half == 0:
                P.cur_qk_ps[m] = P.b1_ps.tile([128, 512], F32, tag="b1", name="qkps")
            ps = P.cur_qk_ps[m]
            for k in range(4 * half, 4 * half + 4):
                nc.tensor.matmul(
                    ps,
                    P.w_sb[:, k, m * 128 : (m + 1) * 128],
                    P.xT_sb[:, k, n * 512 : (n + 1) * 512],
                    start=(k == 0),
                    stop=(k == NDM - 1),
                )
            if half == 1:
                nc.vector.tensor_copy(
                    out=P.qkT[:, m, n * 512 : (n + 1) * 512], in_=ps
                )

        return emit

    def v_half(t4, half):
        def emit():
            t = n * 4 + t4
            if half == 0:
                P.cur_v_ps[t4] = P.b1_ps.tile([128, 512], F32, tag="b1", name="vps")
            ps = P.cur_v_ps[t4]
            for k in range(4 * half, 4 * half + 4):
                nc.tensor.matmul(
                    ps,
                    P.xT_sb[:, k, t * 128 : (t + 1) * 128],
                    P.w_sb[:, k, 2 * GD : 3 * GD],
                    start=(k == 0),
                    stop=(k == NDM - 1),
                )
            if half == 1:
                nc.vector.tensor_copy(
                    out=P.v_sb[:, t, :, 0:HD],
                    in_=ps.rearrange("p (h d) -> p h d", h=HL),
                )

        return emit

    for m in range(2 * GD // 128):
        for half in range(2):
            units.append(qk_half(m, half))
    for t4 in range(4):
        for half in range(2):
            units.append(v_half(t4, half))
    return units


def _attn_segments(nc, P, j):
    """Attention units for q-tile j, one unit list per head pair:
    pipelined S/A + epilogue."""
    segs = []
    units = []
    L = 4 * j + 4
    for hp in range(HL // 2):
        yps = {}
        state = {}

        def alloc(hp=hp, yps=yps):
            for hi in range(2):
                yps[hi] = P.y_ps.tile(
                    [128, 512], F32, tag=f"yps{hi}", name=f"yps{hi}", bufs=1
                )

        def S(c, hp=hp, state=state):
            def emit():
                d = c - 4 * j  # >= 0 on the diagonal band
                off = max(d, 0) * 128  # columns below off are fully masked
                sps2 = P.attn_ps.tile([128, 2, 512], F32, tag="sps2", name="sps2")
                for hi in range(2):
                    po = hi * 64
                    nc.tensor.matmul(
                        sps2[:, hi, off:512],
                        P.qkT[po : po + 64, 4 + hp, c * 128 : (c + 1) * 128],
                        P.qkT[po : po + 64, hp, j * 512 + off : (j + 1) * 512],
                        start=True,
                        stop=True,
                    )
                probs2 = P.probs_p.tile([128, 2, 512], BF16, tag="probs", name="probs")
                nc.scalar.activation(
                    out=probs2[:, :, off:512],
                    in_=sps2[:, :, off:512],
                    func=mybir.ActivationFunctionType.Exp,
                    scale=0.125,
                )
                if d >= 0:
                    # apply the diagonal causal band here so the DVE drains
                    # it early, keeping its queue clear for the epilogue's
                    # PSUM-freeing copies
                    nc.vector.tensor_mul(
                        probs2[:, :, off : off + 128],
                        probs2[:, :, off : off + 128],
                        P.mask_sb,
                    )
                state[c] = (probs2, off, d)

            return emit

        def A(c, hp=hp, yps=yps, state=state):
            def emit():
                probs2, off, d = state.pop(c)
                for hi in range(2):
                    h = 2 * hp + hi
                    # v col 0 is ones -> yps row 0 is the softmax denominator
                    nc.tensor.matmul(
                        yps[hi][0 : HD + 1, off:512],
                        P.v_sb[:, c, h, 0 : HD + 1],
                        probs2[:, hi, off:512],
                        start=(c == 0),
                        stop=(c == L - 1),
                    )

            return emit

        def epilogue(hp=hp, yps=yps, j=j):
            # yps -> SBUF fast (frees PSUM), then normalize into bf16 yT,
            # then ship this head-pair's y chunk through the pair AllGather.
            for hi in range(2):
                po = hi * 64
                # one DVE copy (y rows + den row) frees the PSUM accumulator;
                # den extraction then runs off the critical path on ACT
                yraw = P.yraw_p.tile(
                    [HD + 1, 512], F32, tag=f"yraw{hi}", name=f"yraw{hi}"
                )
                nc.vector.tensor_copy(out=yraw, in_=yps[hi][0 : HD + 1, :])
                den = P.den_p.tile([1, 512], F32, tag=f"den{hi}", name="den")
                nc.scalar.activation(
                    out=den,
                    in_=yraw[HD : HD + 1, :],
                    func=mybir.ActivationFunctionType.Copy,
                )
                nc.vector.reciprocal_approx_fast(out=den, in_=den)
                denb = P.den_p.tile([HD, 512], F32, tag=f"denb{hi}", name="denb")
                nc.gpsimd.partition_broadcast(denb, den)
                nc.vector.tensor_mul(
                    P.yT[po : po + 64, hp, j * 512 : (j + 1) * 512],
                    yraw[0:HD, :],
                    denb,
                )
            idx = 4 * j + hp
            lo, hi_ = j * 512, (j + 1) * 512
            nc.sync.dma_start(out=P.cc_in[idx], in_=P.yT[:, hp, lo:hi_])
            nc.gpsimd.collective_compute(
                "AllGather",
                mybir.AluOpType.bypass,
                replica_groups=[[0, 1], [2, 3], [4, 5], [6, 7]],
                ins=[P.cc_in[idx].opt()],
                outs=[P.cc_out[idx].opt()],
            )
            nc.sync.dma_start(out=P.yF[:, hp, lo:hi_], in_=P.cc_out[idx, 0:128, :])
            nc.sync.dma_start(
                out=P.yF[:, 4 + hp, lo:hi_], in_=P.cc_out[idx, 128:256, :]
            )

        units.append(alloc)
        units.append(S(0))
        for c in range(1, L):
            units.append(S(c))
            units.append(A(c - 1))
        units.append(A(L - 1))
        units.append(epilogue)
        segs.append(units)
        units = []
    return segs


def _proj_units(nc, P, j):
    """Projection units for the 4 token tiles of q-tile j (needs yF[.., j])."""
    if j < 0:
        return []
    units = []
    for mt in range(4 * j, 4 * j + 4):
        st = {}

        def half0(mt=mt, st=st):
            st["ps"] = P.b1_ps.tile([128, 512], F32, tag="b1", name="ops")
            for kk in range(4):
                nc.tensor.matmul(
                    st["ps"],
                    P.yF[:, kk, mt * 128 : (mt + 1) * 128],
                    P.wp_sb[:, kk, :],
                    start=(kk == 0),
                    stop=False,
                )

        def half1(mt=mt, st=st):
            for kk in range(4, 8):
                nc.tensor.matmul(
                    st["ps"],
                    P.yF[:, kk, mt * 128 : (mt + 1) * 128],
                    P.wp_sb[:, kk, :],
                    start=False,
                    stop=(kk == 7),
                )
            acc = P.acc_sb[mt % 4]
            nc.vector.tensor_copy(out=acc, in_=st["ps"])
            nc.sync.dma_start(out=P.out[mt * 128 : (mt + 1) * 128, :], in_=acc)

        units.append(half0)
        units.append(half1)
    return units


def _proj_partial_units(nc, P, j, hp):
    """Tail projection for q-tile j: the two yF chunks unlocked by
    AllGather (j, hp), accumulated into SBUF f32 so the last chunk's work
    after the final AllGather is tiny."""
    units = []
    for mt in range(4 * j, 4 * j + 4):
        def unit(mt=mt, hp=hp):
            ps = P.b1_ps.tile([128, 512], F32, tag="b1", name="pps")
            for kk in (hp, 4 + hp):
                nc.tensor.matmul(
                    ps,
                    P.yF[:, kk, mt * 128 : (mt + 1) * 128],
                    P.wp_sb[:, kk, :],
                    start=(kk == hp),
                    stop=(kk != hp),
                )
            acc = P.acc_sb[mt - 4 * j]
            if hp == 0:
                nc.vector.tensor_copy(out=acc, in_=ps)
            else:
                nc.vector.tensor_add(acc, acc, ps)
            if hp == 3:
                nc.sync.dma_start(
                    out=P.out[mt * 128 : (mt + 1) * 128, :], in_=acc
                )

        units.append(unit)
    return units


class _Ctx:
    pass


def _build_nc():
    nc = bacc.Bacc(None, num_devices=8)
    P = _Ctx()

    xT = nc.dram_tensor("xT", [D, S], BF16, kind="ExternalInput").ap()
    wqkv = nc.dram_tensor("wqkv", [D, 3 * GD], BF16, kind="ExternalInput").ap()
    wproj = nc.dram_tensor("wproj", [D, GD], BF16, kind="ExternalInput").ap()
    masks = nc.dram_tensor("masks", [128, 2, 128], BF16, kind="ExternalInput").ap()
    P.out = nc.dram_tensor("out", [S, GD], F32, kind="ExternalOutput").ap()

    with tile.TileContext(nc) as tc:
        with (
            tc.tile_pool(name="const", bufs=1) as const,
            tc.tile_pool(name="w_p", bufs=1) as w_p,
            tc.tile_pool(name="big_p", bufs=1) as big_p,
            tc.tile_pool(name="probs_p", bufs=6) as probs_p,
            tc.tile_pool(name="yraw_p", bufs=2) as yraw_p,
            tc.tile_pool(name="den_p", bufs=2) as den_p,
            tc.tile_pool(name="out_p", bufs=2) as out_p,
            tc.tile_pool(name="b1_ps", bufs=2, space="PSUM") as b1_ps,
            tc.tile_pool(name="attn_ps", bufs=2, space="PSUM") as attn_ps,
            tc.tile_pool(name="y_ps", bufs=1, space="PSUM") as y_ps,
            tc.tile_pool(name="dram", bufs=1, space="DRAM") as dram,
        ):
            P.probs_p, P.yraw_p, P.den_p, P.out_p = probs_p, yraw_p, den_p, out_p
            P.b1_ps, P.attn_ps, P.y_ps = b1_ps, attn_ps, y_ps
            P.cur_qk_ps, P.cur_v_ps = {}, {}

            # Startup: all plain DMAs, split per chunk and ordered so the
            # first QKV chain can start as soon as x tile 0 + w chunk 0 land.
            P.xT_sb = big_p.tile([128, NDM, S], BF16, name="xT_sb")
            xr = xT.rearrange("(k p) t -> p k t", p=128)
            nc.sync.dma_start(out=P.xT_sb[:, :, 0:512], in_=xr[:, :, 0:512])
            # weights dispatch from the Scalar HWDGE queue, x from Sync, so
            # the ~0.8us-per-trigger dispatch costs overlap
            P.w_sb = w_p.tile([128, NDM, 3 * GD], BF16, name="w_sb")
            wr = wqkv.rearrange("(k p) c -> p k c", p=128)
            for k in range(NDM):
                nc.scalar.dma_start(out=P.w_sb[:, k, :], in_=wr[:, k, :])
            P.mask_sb = const.tile([128, 2, 128], BF16, name="mask_sb")
            nc.scalar.dma_start(out=P.mask_sb, in_=masks)
            for nn in range(1, NQT):
                nc.sync.dma_start(
                    out=P.xT_sb[:, :, nn * 512 : (nn + 1) * 512],
                    in_=xr[:, :, nn * 512 : (nn + 1) * 512],
                )
            P.wp_sb = w_p.tile([128, NDM, GD], BF16, name="wp_sb")
            nc.scalar.dma_start(
                out=P.wp_sb, in_=wproj.rearrange("(k p) c -> p k c", p=128)
            )

            P.qkT = big_p.tile([128, 2 * GD // 128, S], BF16, name="qkT")
            P.v_sb = big_p.tile([128, NTOK, HL, VW], BF16, name="v_sb")
            nc.vector.memset(P.v_sb[:, :, :, HD : HD + 1], 1.0)
            P.yT = big_p.tile([128, GD // 128, S], BF16, name="yT")
            P.yF = big_p.tile([128, NDM, S], BF16, name="yF")

            P.cc_in = dram.tile([4 * NQT, 128, 512], BF16, name="cc_in")
            P.cc_out = dram.tile([4 * NQT, 256, 512], BF16, name="cc_out")

            P.acc_sb = [
                out_p.tile([128, GD], F32, tag=f"acc{i}", name=f"acc{i}", bufs=1)
                for i in range(4)
            ]

            def emit_paced(a_units, f_units):
                na, nf = len(a_units), len(f_units)
                fi = 0
                for i, u in enumerate(a_units):
                    u()
                    while fi < nf and fi * na < (i + 1) * nf:
                        f_units[fi]()
                        fi += 1
                for u in f_units[fi:]:
                    u()

            for u in _qkv_units(nc, P, 0):
                u()
            for n in range(NQT - 1):
                a_units = [u for seg in _attn_segments(nc, P, n) for u in seg]
                f_units = _qkv_units(nc, P, n + 1) + _proj_units(nc, P, n - 1)
                emit_paced(a_units, f_units)
            # Last q-tile: per-segment fills so each head-pair's projection
            # share starts as soon as its AllGather lands.
            segs = _attn_segments(nc, P, NQT - 1)
            emit_paced(segs[0], _proj_units(nc, P, NQT - 2))
            for hp in range(1, 4):
                emit_paced(segs[hp], _proj_partial_units(nc, P, NQT - 1, hp - 1))
            for u in _proj_partial_units(nc, P, NQT - 1, 3):
                u()

    nc.compile()
    return nc


def _host_consts():
    ki = np.arange(128)[:, None]
    qj = np.arange(128)[None, :]
    band = (qj >= ki).astype(ml_dtypes.bfloat16)  # [128, 128] diagonal band
    masks = np.ascontiguousarray(
        np.broadcast_to(band[:, None, :], (128, 2, 128))
    )
    return masks


def _in_maps(x, w_qkv, w_proj):
    masks = _host_consts()
    maps = []
    for c in range(8):
        b, g = c // 2, c % 2
        wq = w_qkv[:, g * GD : (g + 1) * GD]
        wk = w_qkv[:, D + g * GD : D + (g + 1) * GD]
        wv = w_qkv[:, 2 * D + g * GD : 2 * D + (g + 1) * GD]
        maps.append(
            {
                "xT": np.ascontiguousarray(x[b].T).astype(ml_dtypes.bfloat16),
                "wqkv": np.ascontiguousarray(
                    np.concatenate([wq, wk, wv], axis=1)
                ).astype(ml_dtypes.bfloat16),
                "wproj": np.ascontiguousarray(
                    w_proj[:, g * GD : (g + 1) * GD]
                ).astype(ml_dtypes.bfloat16),
                "masks": masks,
            }
        )
    return maps


def kernel(x, w_qkv, w_proj):
    x = np.ascontiguousarray(x, dtype=np.float32)
    w_qkv = np.ascontiguousarray(w_qkv, dtype=np.float32)
    w_proj = np.ascontiguousarray(w_proj, dtype=np.float32)
    if "nc" not in _NC_CACHE:
        _NC_CACHE["nc"] = _build_nc()
    nc = _NC_CACHE["nc"]
    r = run_bass_kernel_spmd(nc, _in_maps(x, w_qkv, w_proj), list(range(8)))
    out = np.empty((4, S, D), dtype=np.float32)
    for b in range(4):
        out[b, :, 0:GD] = r.results[2 * b]["out"]
        out[b, :, GD:D] = r.results[2 * b + 1]["out"]
    return out
